# revision 30
# baseline (speedup 1.0000x reference)

"""Causal attention (no head split) on 8 trn2 NeuronCores.

Reference computation (per batch b):
    q = x @ Wq^T ; k = x @ Wk^T ; v = x @ Wv^T          (nn.Linear convention)
    wei = softmax(mask(q @ k^T / sqrt(C)))               (causal)
    out = wei @ v

Algebraic restructuring (K and V are never materialized):
    S   = q k^T = x (Wq^T Wk) x^T = x M x^T     with M precomputed on host
    out = wei v = (wei x) Wv^T, i.e. O^T = Wv (x^T wei^T) = Wv H
so the device only computes:
    G^T = M^T xq^T                  (one projection of this core's queries)
    S^T[s,t] = x^T(lhsT) G^T(rhs)   (contract over C)
    P^T = exp(S^T / 32) * mask ; rowsum[t] += ones^T P^T
    H[c,t] += x(lhsT) P^T(rhs)      (contract over s, accumulated in SBUF)
    O^T = Wv^T-projection of H      (once per finished query strip)
Final softmax normalization (divide by rowsum) happens on the host.

Sharding: 2 cores per batch (B=4). Queries split into eight 256-row strips;
role A takes strips {0,2,4,6} (rows [512j,512j+256)), role B {1,3,5,7}.
Every core runs the IDENTICAL instruction stream (single SPMD NEFF); role
differences are carried entirely by input data (query columns + mask tiles).
Everything is fp32r (e8m11) on the PE: bf16 speed, ~8x bf16 precision.
"""
import os
import numpy as np

import concourse.bass as bass
from concourse import bacc
import concourse.mybir as mybir
from concourse.tile import TileContext
from concourse import bass_utils

B, T, C = 4, 2048, 1024
P = 128
CS = C // P          # 8 contraction subtiles
NCH = T // 256       # 8 kv chunks of 256
QS = 4               # query strips per core
SW = 256             # strip width
SCALE = 1.0 / np.sqrt(C)  # 1/32

F32R = mybir.dt.float32r
F32 = mybir.dt.float32


def round_fp32r(x: np.ndarray) -> np.ndarray:
    """Round fp32 to fp32r (e8m11): round-to-nearest-even to 11 mantissa bits."""
    x = np.ascontiguousarray(x, dtype=np.float32)
    bits = x.view(np.uint32)
    lsb = (bits >> 12) & 1
    out = (bits + 0x7FF + lsb) & np.uint32(0xFFFFF000)
    return out.view(np.float32)


def build():
    nc = bacc.Bacc(trn_type="TRN2", name="causal_attn")
    xT = nc.dram_tensor("xT", [C, T], F32R, kind="ExternalInput")    # x^T (batch)
    xn = nc.dram_tensor("xn", [T, C], F32R, kind="ExternalInput")    # x natural
    xqT = nc.dram_tensor("xqT", [C, QS * SW], F32R, kind="ExternalInput")
    wm = nc.dram_tensor("wm", [C, C], F32R, kind="ExternalInput")    # M = Wq^T Wk
    wvT = nc.dram_tensor("wvT", [C, C], F32R, kind="ExternalInput")  # Wv^T [c,d]
    masks = nc.dram_tensor("masks", [P, 4, SW], F32R, kind="ExternalInput")
    ones = nc.dram_tensor("ones", [P, 1], F32R, kind="ExternalInput")
    outT = nc.dram_tensor("outT", [C, QS * SW], F32, kind="ExternalOutput")
    rows = nc.dram_tensor("rows", [1, QS * SW], F32, kind="ExternalOutput")

    xT_r = xT.rearrange("(cs p) t -> p cs t", p=P)
    xn_r = xn.rearrange("(ch ss p) c -> p ch ss c", p=P, ss=2)
    xqT_r = xqT.rearrange("(cs p) t -> p cs t", p=P)
    wm_r = wm.rearrange("(cs p) d -> p cs d", p=P)
    wvT_r = wvT.rearrange("(cs p) d -> p cs d", p=P)
    outT_r = outT.rearrange("(ds p) t -> p ds t", p=P)
    rows_r = rows.rearrange("p (a b) -> p a b", a=QS)

    with TileContext(nc) as tc:
        with tc.tile_pool(name="keep", bufs=1) as keep, \
             tc.tile_pool(name="wpool", bufs=2) as wpool, \
             tc.tile_pool(name="stream", bufs=2) as stream, \
             tc.tile_pool(name="hrpool", bufs=2) as hrpool, \
             tc.tile_pool(name="ppool", bufs=3) as ppool, \
             tc.tile_pool(name="psA", bufs=2, space="PSUM") as psA, \
             tc.tile_pool(name="psS", bufs=3, space="PSUM") as psS, \
             tc.tile_pool(name="psO", bufs=2, space="PSUM") as psO, \
             tc.tile_pool(name="psR", bufs=1, space="PSUM") as psR:

            gT = keep.tile([P, CS, QS * SW], F32R, tag="gT")   # G^T  32KB/part
            hh = keep.tile([P, CS, QS * SW], F32, tag="hh")    # H    32KB/part
            msk = keep.tile([P, 4, SW], F32R, tag="msk")
            ones_t = keep.tile([P, 1], F32R, tag="ones")
            rowsum = keep.tile([1, QS, SW], F32, tag="rowsum")
            # ---- Phase G: G^T = M^T xq^T for the 4 query strips ----
            # first group's weight slice goes out first (longest pole), split
            # in halves across two queues; then the strip-0 queries.
            wq = wpool.tile([P, CS, C], F32R, tag="w")
            for h in range(2):
                nc.sync.dma_start(wq[:, 4 * h:4 * h + 4, 0:P],
                                  wm_r[:, 4 * h:4 * h + 4, 0:P])
            xq0 = stream.tile([P, CS, SW], F32R, tag="xt")
            for h in range(4):
                nc.sync.dma_start(
                    xq0[:, 2 * h:2 * h + 2],
                    xqT_r[:, 2 * h:2 * h + 2, 0:SW])
            for ds in range(1, CS):
                nc.sync.dma_start(wq[:, :, ds * P:(ds + 1) * P],
                                  wm_r[:, :, ds * P:(ds + 1) * P])
            nc.sync.dma_start(msk[:], masks[:])
            nc.sync.dma_start(ones_t[:], ones[:])
            for j in range(QS):
                if j == 0:
                    xq = xq0
                else:
                    xq = stream.tile([P, CS, SW], F32R, tag="xt")
                    for h in range(4):
                        nc.sync.dma_start(
                            xq[:, 2 * h:2 * h + 2],
                            xqT_r[:, 2 * h:2 * h + 2, j * SW:(j + 1) * SW])
                for ds in range(CS):
                    pq = psA.tile([P, SW], F32, tag="prod")
                    for cs in range(CS):
                        nc.tensor.matmul(
                            pq[:], wq[:, cs, ds * P:(ds + 1) * P], xq[:, cs],
                            start=(cs == 0), stop=(cs == CS - 1))
                    nc.scalar.copy(gT[:, ds, j * SW:(j + 1) * SW], pq[:])

            # Wv^T for the final output projections (second w slot)
            wv = wpool.tile([P, CS, C], F32R, tag="w")
            for dh in range(2):
                nc.sync.dma_start(wv[:, :, dh * 512:(dh + 1) * 512],
                                  wvT_r[:, :, dh * 512:(dh + 1) * 512])

            # ---- Chunk loop: stream x^T / x for chunk c, attend all strips.
            # Order ends at chunks 4,5 so strips 2 AND 3 both complete near the
            # end and their Wv-projections interleave (fills the tail chain).
            CHUNK_ORDER = [0, 1, 2, 3, 6, 7, 4, 5]
            LAST_VISIT = {j: max(range(NCH), key=lambda p: (CHUNK_ORDER[p] <= 2 * j + 1, p))
                          for j in range(QS)}
            for pos in range(NCH):
                c = CHUNK_ORDER[pos]
                xt = stream.tile([P, CS, 256], F32R, tag="xt")
                for h in range(4):
                    nc.sync.dma_start(
                        xt[:, 2 * h:2 * h + 2],
                        xT_r[:, 2 * h:2 * h + 2, c * 256:(c + 1) * 256])
                xna = stream.tile([P, 2, C], F32R, tag="xn")
                for ss in range(2):
                    for h in range(2):
                        nc.sync.dma_start(
                            xna[:, ss, h * 512:(h + 1) * 512],
                            xn_r[:, c, ss, h * 512:(h + 1) * 512])

                # strips that attend to chunk c: 2j+1 >= c
                for j in range(QS):
                    if 2 * j + 1 < c:
                        continue
                    tsl = slice(j * SW, (j + 1) * SW)

                    st = psS.tile([P, 2, SW], F32, tag="st")
                    for ss in range(2):
                        for cs in range(CS):
                            nc.tensor.matmul(
                                st[:, ss], xt[:, cs, ss * P:(ss + 1) * P],
                                gT[:, cs, tsl],
                                start=(cs == 0), stop=(cs == CS - 1))

                    pT = ppool.tile([P, 2, SW], F32R, tag="pT")
                    nc.scalar.activation(
                        pT[:], st[:],
                        mybir.ActivationFunctionType.Exp, scale=float(SCALE))

                    midx = None
                    if c == 2 * j:
                        midx = 0
                    elif c == 2 * j + 1:
                        midx = 1
                    if midx is not None:
                        nc.vector.tensor_mul(
                            pT[:], pT[:], msk[:, midx * 2:midx * 2 + 2])

                    rw = psR.tile([1, SW], F32, tag="rw")
                    for ss in range(2):
                        nc.tensor.matmul(
                            rw[:], ones_t[:], pT[:, ss],
                            start=(ss == 0), stop=(ss == 1))
                    if c == 0:
                        nc.vector.tensor_copy(rowsum[:, j], rw[:])
                    else:
                        nc.vector.tensor_add(rowsum[:, j], rowsum[:, j], rw[:])

                    # H[c,t] += x(lhsT) @ P^T, c-subtiles in quarters
                    for q4 in range(4):
                        po = psO.tile([P, 2, SW], F32, tag="po")
                        for i in range(2):
                            cs4 = 2 * q4 + i
                            for ss in range(2):
                                nc.tensor.matmul(
                                    po[:, i], xna[:, ss, cs4 * P:(cs4 + 1) * P],
                                    pT[:, ss],
                                    start=(ss == 0), stop=(ss == 1))
                        hsl = hh[:, 2 * q4:2 * q4 + 2, tsl]
                        if c == 0:
                            nc.vector.tensor_copy(hsl, po[:])
                        else:
                            nc.vector.tensor_add(hsl, hsl, po[:])

                    # strip complete after its last chunk: project by Wv^T.
                    # hr cast per c-half so the projection's early contraction
                    # steps overlap the tail of H accumulation.
                    if pos == LAST_VISIT[j]:
                        hr = hrpool.tile([P, CS, SW], F32R, tag="hr")
                        for q2 in range(2):
                            nc.scalar.copy(hr[:, 4 * q2:4 * q2 + 4],
                                           hh[:, 4 * q2:4 * q2 + 4, tsl])
                        ost = hrpool.tile([P, CS, SW], F32, tag="ost")
                        for ds in range(CS):
                            pf = psA.tile([P, SW], F32, tag="prod")
                            for cs in range(CS):
                                nc.tensor.matmul(
                                    pf[:], wv[:, cs, ds * P:(ds + 1) * P],
                                    hr[:, cs],
                                    start=(cs == 0), stop=(cs == CS - 1))
                            nc.vector.tensor_copy(ost[:, ds], pf[:])
                            nc.sync.dma_start(outT_r[:, ds, tsl], ost[:, ds])

            nc.sync.dma_start(rows_r[:], rowsum[:])

    nc.compile()
    return nc


_NC = None


def _get_nc():
    global _NC
    if _NC is None:
        _NC = build()
    return _NC


def make_in_maps(x, Wq, Wk, Wv):
    x = np.asarray(x, dtype=np.float32)
    wq64 = np.asarray(Wq, np.float64)
    wk64 = np.asarray(Wk, np.float64)
    wm = round_fp32r((wq64.T @ wk64).astype(np.float32))     # M = Wq^T Wk [c',c]
    wvT = round_fp32r(np.asarray(Wv, np.float32).T)
    ones = np.ones((P, 1), np.float32)

    # mask tiles [p, midx*2+ss, t]: tri = 1 if (ss*128+p) <= t
    s_idx = (np.arange(2)[:, None, None] * P + np.arange(P)[None, :, None])
    tri = (s_idx <= np.arange(SW)[None, None, :]).astype(np.float32)
    tri = np.ascontiguousarray(tri.transpose(1, 0, 2))
    zeros = np.zeros((P, 2, SW), np.float32)
    ones2 = np.ones((P, 2, SW), np.float32)
    mask_A = np.ascontiguousarray(np.concatenate([tri, zeros], axis=1), np.float32)
    mask_B = np.ascontiguousarray(np.concatenate([ones2, tri], axis=1), np.float32)

    xr = [round_fp32r(x[b]) for b in range(B)]
    xT = [np.ascontiguousarray(xr[b].T) for b in range(B)]
    in_maps = []
    for core in range(8):
        b, role = divmod(core, 2)
        cols = np.concatenate(
            [np.arange(512 * j + SW * role, 512 * j + SW * role + SW)
             for j in range(QS)])
        xqT = np.ascontiguousarray(xT[b][:, cols])
        in_maps.append({
            "xT": xT[b],
            "xn": xr[b],
            "xqT": xqT,
            "wm": wm, "wvT": wvT,
            "masks": mask_A if role == 0 else mask_B,
            "ones": ones,
        })
    return in_maps


def assemble(results):
    out = np.empty((B, T, C), np.float32)
    for core in range(8):
        b, role = divmod(core, 2)
        oT = results[core]["outT"]                   # [C, 1024]
        rsum = results[core]["rows"].reshape(QS * SW)
        o = oT.T / rsum[:, None]
        for j in range(QS):
            r0 = 512 * j + SW * role
            out[b, r0:r0 + SW] = o[j * SW:(j + 1) * SW]
    return out


def kernel(x, Wq, Wk, Wv):
    nc = _get_nc()
    in_maps = make_in_maps(x, Wq, Wk, Wv)
    res = bass_utils.run_bass_kernel_spmd(nc, in_maps, core_ids=list(range(8)))
    return assemble(res.results)


def _install_trace_shim():
    """Provide antenv.axon_hooks (absent in this image) so trace=True works."""
    import sys
    import types
    if "antenv.axon_hooks" in sys.modules:
        return
    hook_box = [None]
    mod = types.ModuleType("antenv.axon_hooks")
    mod.set_axon_ntff_profile_hook = lambda h: hook_box.__setitem__(0, h)
    mod.get_axon_ntff_profile_hook = lambda: hook_box[0]
    import antenv
    sys.modules["antenv.axon_hooks"] = mod
    antenv.axon_hooks = mod
    try:
        from trn_agent_boot.trn_boot import _ntff_profile_via_ctypes
        mod.set_axon_ntff_profile_hook(
            _ntff_profile_via_ctypes("/opt/axon/libaxon_pjrt.so"))
    except Exception:
        pass


def run_traced(x, Wq, Wk, Wv):
    """Like kernel() but with NTFF tracing; returns (out, BassKernelResults)."""
    _install_trace_shim()
    nc = _get_nc()
    in_maps = make_in_maps(x, Wq, Wk, Wv)
    res = bass_utils.run_bass_kernel_spmd(
        nc, in_maps, core_ids=list(range(8)), trace=True,
        trace_cores=list(range(8)))
    return assemble(res.results), res
